# revision 1
# baseline (speedup 1.0000x reference)
"""LinearAttention Trainium2 kernel: data-parallel over batch on 8 NeuronCores.

Reference computation per batch b (C=256 channels, L=4096 seq, H=8 heads, D=64):
  qkv = w_qkv @ x[b]                    # (1536, L)
  q, k, v = split(qkv)                  # each (512, L), rows = (head, dim)
  k = softmax(k, axis=L)
  ctx[h] = k[h] @ v[h].T                # (64, 64)
  out[h] = ctx[h].T @ q[h]              # (64, L)
  y[b] = w_out @ concat(out) + b_out    # (256, L)

Per-core design (2 batches/core):
  - K^T, V^T computed with L on partitions (lhsT = x chunk, rhs = w^T) so the
    context matmul contracts over L on the TensorEngine.
  - context computed TRANSPOSED per head-pair: ctxT[e,d] = sum_l v[e,l]exp(k[d,l])
    (lhsT = v^T chunk, rhs = expk^T chunk), cross-head quadrants discarded via
    a zeroed block-diagonal SBUF tile.
  - w_out is folded into the context on the PE: McT[d,o] = sum_e ctxT[e,d]wo[e,o],
    which removes the separate attention-out matmul and its PSUM->SBUF copies.
    The softmax denominator (row matmul with a ones lhsT, then 4 tiny PE
    transposes) is applied as a per-partition ACT scale on the McT copy.
  - final: y = McT.T @ q + b, contracting the 512 q-channels in 4 chunks.
  - exp() applied unshifted (inputs are N(0,1)-scaled; max |k| ~ 5, safe in f32).
  - all TensorE compute in bf16 (f32 PSUM accumulation).
"""

import numpy as np

B, C, L = 16, 256, 4096
HID = 512
N_CORES = 8
NB = B // N_CORES  # batches per core
CC = C // 128  # contraction chunks for the input projections (2)
LP = L // 128  # l-tiles with l on partitions (32)
LT = L // 512  # l-tiles of 512 for moving-dim matmuls (8)
PR = HID // 128  # head-pairs (4): each 128-wide chunk = 2 heads of 64

_CACHE = {}


def _build(reps=1):
    from concourse import bacc, mybir, tile
    import concourse.bass as bass

    bf16 = mybir.dt.bfloat16
    f32 = mybir.dt.float32
    Exp = mybir.ActivationFunctionType.Exp
    Copy = mybir.ActivationFunctionType.Copy
    Ident = mybir.ActivationFunctionType.Identity

    nc = bacc.Bacc(
        "TRN2",
        target_bir_lowering=False,
        debug=False,
        enable_asserts=False,
        num_devices=N_CORES,
    )

    x_d = nc.dram_tensor("x", [NB, CC, 128, L], bf16, kind="ExternalInput")
    wq_d = nc.dram_tensor("wq_t", [CC, 128, HID], bf16, kind="ExternalInput")
    wk_d = nc.dram_tensor("wk_t", [CC, 128, HID], bf16, kind="ExternalInput")
    wv_d = nc.dram_tensor("wv_t", [CC, 128, HID], bf16, kind="ExternalInput")
    wo_d = nc.dram_tensor("wo_t", [PR, 128, C], bf16, kind="ExternalInput")
    bb_d = nc.dram_tensor("bb", [128, 2], f32, kind="ExternalInput")
    out_d = nc.dram_tensor("out", [NB, 2, 128, L], f32, kind="ExternalOutput")

    with tile.TileContext(nc) as tc:
        with (
            tc.tile_pool(name="const", bufs=1) as const,
            tc.tile_pool(name="xp", bufs=2) as xp,
            tc.tile_pool(name="big", bufs=1) as big,
            tc.tile_pool(name="small", bufs=2) as small,
            tc.tile_pool(name="qtp", bufs=4) as qtp,
            tc.tile_pool(name="ostp", bufs=3) as ostp,
            tc.tile_pool(name="ps_mm", bufs=3, space="PSUM") as ps_mm,
            tc.tile_pool(name="ps_ctx", bufs=4, space="PSUM") as ps_ctx,
            tc.tile_pool(name="ps_den", bufs=1, space="PSUM") as ps_den,
        ):
            wq = const.tile([128, CC, HID], bf16)
            wk = const.tile([128, CC, HID], bf16)
            wv = const.tile([128, CC, HID], bf16)
            wo = const.tile([128, PR, C], bf16)
            bb = const.tile([128, 2], f32)
            ones_col = const.tile([128, 1], bf16)
            id11 = const.tile([1, 1], f32)
            ctxt_sb = const.tile([128, PR, 128], bf16)

            for cc in range(CC):
                nc.sync.dma_start(wq[:, cc, :], wq_d[cc])
                nc.sync.dma_start(wk[:, cc, :], wk_d[cc])
                nc.sync.dma_start(wv[:, cc, :], wv_d[cc])
            for pr in range(PR):
                nc.sync.dma_start(wo[:, pr, :], wo_d[pr])
            nc.sync.dma_start(bb[:], bb_d[:])
            nc.gpsimd.memset(ones_col[:], 1.0)
            nc.gpsimd.memset(id11[:], 1.0)
            nc.gpsimd.memset(ctxt_sb[:], 0.0)

            for rep in range(reps):
              for bi in range(NB):
                xt = xp.tile([128, CC, L], bf16)
                for cc in range(CC):
                    nc.sync.dma_start(xt[:, cc, :], x_d[bi, cc])

                expkt = big.tile([128, LP, HID], bf16, tag="expkt")
                vt = big.tile([128, LP, HID], bf16, tag="vt")

                # K^T / V^T projections fused with the transposed-context and
                # denominator accumulations: PE streams without phase breaks.
                ctx_p = [
                    ps_ctx.tile([128, 128], f32, tag="ctx", name=f"ctx_{rep}_{bi}_{g}")
                    for g in range(PR)
                ]
                den_ps = ps_den.tile([1, HID], f32, tag="den")
                for lp in range(LP):
                    psk = ps_mm.tile([128, HID], f32, tag="mm")
                    psv = ps_mm.tile([128, HID], f32, tag="mm")
                    for cc in range(CC):
                        nc.tensor.matmul(
                            psk[:],
                            xt[:, cc, lp * 128 : (lp + 1) * 128],
                            wk[:, cc, :],
                            start=(cc == 0),
                            stop=(cc == CC - 1),
                        )
                    for cc in range(CC):
                        nc.tensor.matmul(
                            psv[:],
                            xt[:, cc, lp * 128 : (lp + 1) * 128],
                            wv[:, cc, :],
                            start=(cc == 0),
                            stop=(cc == CC - 1),
                        )
                    nc.scalar.activation(expkt[:, lp, :], psk[:], Exp)
                    nc.vector.tensor_copy(vt[:, lp, :], psv[:])
                    for pr in range(PR):
                        nc.tensor.matmul(
                            ctx_p[pr][:],
                            vt[:, lp, pr * 128 : (pr + 1) * 128],
                            expkt[:, lp, pr * 128 : (pr + 1) * 128],
                            start=(lp == 0),
                            stop=(lp == LP - 1),
                        )
                    nc.tensor.matmul(
                        den_ps[:],
                        ones_col[:],
                        expkt[:, lp, :],
                        start=(lp == 0),
                        stop=(lp == LP - 1),
                        skip_group_check=True,
                    )
                den_sb = small.tile([1, HID], f32, tag="densb")
                nc.vector.tensor_copy(den_sb[:], den_ps[:])
                tps = ps_mm.tile([128, PR], f32, tag="mm")
                for pr in range(PR):
                    nc.tensor.transpose(
                        tps[:, pr : pr + 1],
                        den_sb[0:1, pr * 128 : (pr + 1) * 128],
                        id11[:],
                    )
                inv_den = small.tile([128, PR], f32, tag="invden")
                nc.vector.reciprocal(inv_den[:], tps[:])

                # block-diagonal ctxT (cross-head quadrants stay zero).
                for pr in range(PR):
                    nc.vector.tensor_copy(
                        ctxt_sb[0:64, pr, 0:64], ctx_p[pr][0:64, 0:64]
                    )
                    nc.vector.tensor_copy(
                        ctxt_sb[64:128, pr, 64:128], ctx_p[pr][64:128, 64:128]
                    )
                # fold w_out into the context: McT[d, o], scaled by 1/den[d].
                mct = small.tile([128, PR, C], bf16, tag="mct")
                for pr in range(PR):
                    mc_ps = ps_mm.tile([128, C], f32, tag="mm")
                    nc.tensor.matmul(
                        mc_ps[:], ctxt_sb[:, pr, :], wo[:, pr, :], start=True, stop=True
                    )
                    nc.scalar.activation(
                        mct[:, pr, :], mc_ps[:], Copy, scale=inv_den[:, pr : pr + 1]
                    )

                # Q projection + fused output projection, per l-chunk of 512.
                for lt in range(LT):
                    qt = qtp.tile([128, PR, 512], bf16, tag="qt")
                    for oc in range(PR):
                        psq = ps_mm.tile([128, 512], f32, tag="mm")
                        for cc in range(CC):
                            nc.tensor.matmul(
                                psq[:],
                                wq[:, cc, oc * 128 : (oc + 1) * 128],
                                xt[:, cc, lt * 512 : (lt + 1) * 512],
                                start=(cc == 0),
                                stop=(cc == CC - 1),
                            )
                        nc.vector.tensor_copy(qt[:, oc, :], psq[:])
                    ostg = ostp.tile([128, 2, 512], f32, tag="ostg")
                    for oc2 in range(2):
                        psf = ps_mm.tile([128, 512], f32, tag="mm")
                        for pr in range(PR):
                            nc.tensor.matmul(
                                psf[:],
                                mct[:, pr, oc2 * 128 : (oc2 + 1) * 128],
                                qt[:, pr, :],
                                start=(pr == 0),
                                stop=(pr == PR - 1),
                            )
                        nc.scalar.activation(
                            ostg[:, oc2, :],
                            psf[:],
                            Ident,
                            bias=bb[:, oc2 : oc2 + 1],
                        )
                        nc.sync.dma_start(
                            out_d[bi, oc2, :, lt * 512 : (lt + 1) * 512],
                            ostg[:, oc2, :],
                        )

    nc.compile()
    return nc


def _get_nc():
    if "nc" not in _CACHE:
        _CACHE["nc"] = _build()
    return _CACHE["nc"]


def _prep_in_maps(x, w_qkv, w_out, b_out):
    import ml_dtypes

    bf16 = ml_dtypes.bfloat16
    wq_t = np.ascontiguousarray(w_qkv[0:512].T).reshape(CC, 128, HID).astype(bf16)
    wk_t = np.ascontiguousarray(w_qkv[512:1024].T).reshape(CC, 128, HID).astype(bf16)
    wv_t = np.ascontiguousarray(w_qkv[1024:1536].T).reshape(CC, 128, HID).astype(bf16)
    wo_t = np.ascontiguousarray(w_out.T).reshape(PR, 128, C).astype(bf16)
    bb = np.ascontiguousarray(b_out.reshape(2, 128).T).astype(np.float32)
    in_maps = []
    for c in range(N_CORES):
        xs = x[c * NB : (c + 1) * NB].reshape(NB, CC, 128, L).astype(bf16)
        in_maps.append(
            {
                "x": np.ascontiguousarray(xs),
                "wq_t": wq_t,
                "wk_t": wk_t,
                "wv_t": wv_t,
                "wo_t": wo_t,
                "bb": bb,
            }
        )
    return in_maps


def kernel(x, w_qkv, w_out, b_out):
    from concourse.bass_utils import run_bass_kernel_spmd

    nc = _get_nc()
    in_maps = _prep_in_maps(
        np.asarray(x, dtype=np.float32),
        np.asarray(w_qkv, dtype=np.float32),
        np.asarray(w_out, dtype=np.float32),
        np.asarray(b_out, dtype=np.float32),
    )
    res = run_bass_kernel_spmd(nc, in_maps, core_ids=list(range(N_CORES)))
    out = np.concatenate(
        [res.results[c]["out"].reshape(NB, C, L) for c in range(N_CORES)], axis=0
    )
    return out.astype(np.float32)



# revision 26
# speedup vs baseline: 3.0486x; 3.0486x over previous
"""LinearAttention Trainium2 kernel: data-parallel over batch on 8 NeuronCores.

Reference computation per batch b (C=256 channels, L=4096 seq, H=8 heads, D=64):
  qkv = w_qkv @ x[b]                    # (1536, L)
  q, k, v = split(qkv)                  # each (512, L), rows = (head, dim)
  k = softmax(k, axis=L)
  ctx[h] = k[h] @ v[h].T                # (64, 64)
  out[h] = ctx[h].T @ q[h]              # (64, L)
  y[b] = w_out @ concat(out) + b_out    # (256, L)

Per-core design (2 batches/core):
  - K^T, V^T computed with L on partitions (lhsT = x chunk, rhs = w^T) so the
    context matmul contracts over L on the TensorEngine.
  - context computed TRANSPOSED per head-pair: ctxT[e,d] = sum_l v[e,l]exp(k[d,l])
    (lhsT = v^T chunk, rhs = expk^T chunk), cross-head quadrants discarded via
    a zeroed block-diagonal SBUF tile.
  - softmax denominator via transposed N=1 matmuls per l-tile:
    den[d,1] += expkT_chunk^T @ ones — d lands on partitions directly and each
    matmul runs at the ~60-cycle issue floor instead of streaming 512 columns.
  - PSUM accumulation-group discipline: the has_written clear on start=True is
    bank-wide, so any bank hosting several interleaved accumulation groups
    (the four ctx tiles in one bank, the four den columns in another) is
    opened by a single zero-writing matmul; the real groups then accumulate
    with start=False, WAW-ordered after the clear.
  - two PSUM pools ping-pong between batches so one batch's K/V projections
    can overlap the previous batch's output phase on the PE (a shared pool's
    buffer rotation would serialize them).
  - Q projection fused into the K/V loop (one 512-col stripe per 4 l-tiles)
    with PSUM->SBUF copies alternating between ACT and DVE.
  - w_out is folded into the context on the PE: McT[d,o] = sum_e ctxT[e,d]wo[e,o],
    scaled by 1/den[d] on the PSUM->SBUF copy (split per 128-col half across
    ACT and DVE so the final matmuls start sooner).
  - final: y = McT.T @ q + b, contracting the 512 q-channels in 4 chunks;
    output stored as bf16 (well within tolerance, halves store traffic).
  - x DMA is issued in chunks with the first 512 columns ahead of most weight
    loads; the very last output tile runs in 128-col pieces end-to-end so the
    drain tail after the final matmul is short.
  - exp() applied unshifted (inputs are N(0,1)-scaled; max |k| ~ 5, safe in f32).
  - all TensorE compute in bf16 (f32 PSUM accumulation).
"""

import numpy as np

B, C, L = 16, 256, 4096
HID = 512
N_CORES = 8
NB = B // N_CORES  # batches per core
CC = C // 128  # contraction chunks for the input projections (2)
LP = L // 128  # l-tiles with l on partitions (32)
LT = L // 512  # l-tiles of 512 for moving-dim matmuls (8)
PR = HID // 128  # head-pairs (4): each 128-wide chunk = 2 heads of 64
X_CHUNKS = (512, 512, 1024, 1024, 1024)  # x DMA chunk widths per batch

_CACHE = {}


def _build(reps=1):
    from concourse import bacc, mybir, tile

    bf16 = mybir.dt.bfloat16
    f32 = mybir.dt.float32
    Exp = mybir.ActivationFunctionType.Exp
    Copy = mybir.ActivationFunctionType.Copy
    Ident = mybir.ActivationFunctionType.Identity

    nc = bacc.Bacc(
        "TRN2",
        target_bir_lowering=False,
        debug=False,
        enable_asserts=False,
        num_devices=N_CORES,
    )

    # DRAM layouts put the partition dim first so whole-tensor DMAs iterate
    # in the same (partition-major) order as their SBUF-side APs.
    x_d = nc.dram_tensor("x", [NB, 128, CC, L], bf16, kind="ExternalInput")
    wq_d = nc.dram_tensor("wq_t", [128, CC, HID], bf16, kind="ExternalInput")
    wk_d = nc.dram_tensor("wk_t", [128, CC, HID], bf16, kind="ExternalInput")
    wv_d = nc.dram_tensor("wv_t", [128, CC, HID], bf16, kind="ExternalInput")
    wo_d = nc.dram_tensor("wo_t", [128, PR, C], bf16, kind="ExternalInput")
    bb_d = nc.dram_tensor("bb", [128, 2], f32, kind="ExternalInput")
    out_d = nc.dram_tensor("out", [NB, 128, 2, L], bf16, kind="ExternalOutput")

    with tile.TileContext(nc) as tc:
        with (
            tc.tile_pool(name="const", bufs=1) as const,
            tc.tile_pool(name="xp", bufs=2) as xp,
            tc.tile_pool(name="big", bufs=1) as big,
            tc.tile_pool(name="small", bufs=2) as small,
            tc.tile_pool(name="ostp", bufs=3) as ostp,
            tc.tile_pool(name="ps_a", bufs=3, space="PSUM") as ps_a,
            tc.tile_pool(name="ps_b", bufs=3, space="PSUM") as ps_b,
            tc.tile_pool(name="ps_ctx", bufs=1, space="PSUM") as ps_ctx,
            tc.tile_pool(name="ps_den", bufs=1, space="PSUM") as ps_den,
        ):
            wq = const.tile([128, CC, HID], bf16)
            wk = const.tile([128, CC, HID], bf16)
            wv = const.tile([128, CC, HID], bf16)
            wo = const.tile([128, PR, C], bf16)
            bb = const.tile([128, 2], f32)
            ones_col = const.tile([128, 1], bf16)
            zeros128 = const.tile([128, 128], bf16)
            ctxt_sb = const.tile([128, PR, 128], bf16)

            # Startup DMA order follows first-use time on the serial SP queue.
            xt0 = xp.tile([128, CC, L], bf16, name="xt_first")
            nc.sync.dma_start(xt0[:, :, 0:512], x_d[0, :, :, 0:512])
            nc.sync.dma_start(wk[:], wk_d[:])
            nc.sync.dma_start(wv[:], wv_d[:])
            nc.sync.dma_start(xt0[:, :, 512:1024], x_d[0, :, :, 512:1024])
            nc.sync.dma_start(wq[:], wq_d[:])
            nc.sync.dma_start(xt0[:, :, 1024:2048], x_d[0, :, :, 1024:2048])
            nc.sync.dma_start(wo[:], wo_d[:])
            nc.sync.dma_start(bb[:], bb_d[:])
            nc.sync.dma_start(xt0[:, :, 2048:3072], x_d[0, :, :, 2048:3072])
            nc.sync.dma_start(xt0[:, :, 3072:4096], x_d[0, :, :, 3072:4096])
            nc.gpsimd.memset(ones_col[:], 1.0)
            nc.gpsimd.memset(zeros128[:], 0.0)
            nc.gpsimd.memset(ctxt_sb[:], 0.0)

            def emit_q_stripe(qt, xt, mm_pool, lt):
                for oc in range(PR):
                    psq = mm_pool.tile([128, 512], f32, tag="mm")
                    for cc in range(CC):
                        nc.tensor.matmul(
                            psq[:],
                            wq[:, cc, oc * 128 : (oc + 1) * 128],
                            xt[:, cc, lt * 512 : (lt + 1) * 512],
                            start=(cc == 0),
                            stop=(cc == CC - 1),
                        )
                    dst = qt[:, oc, lt * 512 : (lt + 1) * 512]
                    if oc % 2 == 0:
                        nc.vector.tensor_copy(dst, psq[:])
                    else:
                        nc.scalar.activation(dst, psq[:], Copy)

            def emit_x_dma(xt, bi):
                l0 = 0
                for cw in X_CHUNKS:
                    nc.sync.dma_start(
                        xt[:, :, l0 : l0 + cw], x_d[bi, :, :, l0 : l0 + cw]
                    )
                    l0 += cw

            for rep in range(reps):
              for bi in range(NB):
                first = rep == 0 and bi == 0
                mm_pool = ps_a if (rep * NB + bi) % 2 == 0 else ps_b
                xt = xt0 if first else xp.tile([128, CC, L], bf16)
                if not first:
                    emit_x_dma(xt, bi)

                expkt = big.tile([128, LP, HID], bf16, tag="expkt")
                vt = big.tile([128, LP, HID], bf16, tag="vt")
                qt = big.tile([128, PR, L], bf16, tag="qt")

                # all four ctx tiles live in ONE PSUM bank; zero-writing
                # matmuls open the bank (bank-wide has_written clear on the
                # first), then the real groups accumulate with start=False.
                ctxb = ps_ctx.tile([128, PR, 128], f32, tag="ctx")
                den_ps = ps_den.tile([128, PR], f32, tag="den")
                for pr in range(PR):
                    nc.tensor.matmul(
                        ctxb[:, pr, :],
                        zeros128[:],
                        zeros128[:],
                        start=(pr == 0),
                        stop=True,
                        skip_group_check=True,
                    )
                nc.tensor.matmul(
                    den_ps[:], zeros128[:], zeros128[:, 0:PR], start=True, stop=True
                )
                for lp in range(LP):
                    psk = mm_pool.tile([128, HID], f32, tag="mm")
                    psv = mm_pool.tile([128, HID], f32, tag="mm")
                    for cc in range(CC):
                        nc.tensor.matmul(
                            psk[:],
                            xt[:, cc, lp * 128 : (lp + 1) * 128],
                            wk[:, cc, :],
                            start=(cc == 0),
                            stop=(cc == CC - 1),
                        )
                    for cc in range(CC):
                        nc.tensor.matmul(
                            psv[:],
                            xt[:, cc, lp * 128 : (lp + 1) * 128],
                            wv[:, cc, :],
                            start=(cc == 0),
                            stop=(cc == CC - 1),
                        )
                    nc.scalar.activation(expkt[:, lp, :], psk[:], Exp)
                    if lp < LP - 1:
                        nc.vector.tensor_copy(vt[:, lp, :], psv[:])
                    else:
                        # last tile in 128-col pieces: shortens the gate on the
                        # trailing context matmuls.
                        for pr in range(PR):
                            nc.vector.tensor_copy(
                                vt[:, lp, pr * 128 : (pr + 1) * 128],
                                psv[:, pr * 128 : (pr + 1) * 128],
                            )
                    for pr in range(PR):
                        nc.tensor.matmul(
                            ctxb[:, pr, :],
                            vt[:, lp, pr * 128 : (pr + 1) * 128],
                            expkt[:, lp, pr * 128 : (pr + 1) * 128],
                            start=False,
                            stop=(lp == LP - 1),
                            skip_group_check=True,
                        )
                    for pr in range(PR):
                        nc.tensor.matmul(
                            den_ps[:, pr : pr + 1],
                            expkt[:, lp, pr * 128 : (pr + 1) * 128],
                            ones_col[:],
                            start=False,
                            stop=(lp == LP - 1),
                            skip_group_check=True,
                        )
                    if lp % 4 == 2:
                        # Q projection for one 512-col stripe; copies alternate
                        # DVE/ACT so neither becomes the pipeline drag.
                        emit_q_stripe(qt, xt, mm_pool, lp // 4)

                inv_den = small.tile([128, PR], f32, tag="invden")
                nc.vector.reciprocal(inv_den[:], den_ps[:])

                # block-diagonal ctxT (cross-head quadrants stay zero); the two
                # diagonal blocks per pair go to different engines in parallel.
                for pr in range(PR):
                    nc.vector.tensor_copy(
                        ctxt_sb[0:64, pr, 0:64], ctxb[0:64, pr, 0:64]
                    )
                    nc.scalar.activation(
                        ctxt_sb[64:128, pr, 64:128], ctxb[64:128, pr, 64:128], Copy
                    )

                # fold w_out into the context: McT[d, o], scaled by 1/den[d].
                mct = small.tile([128, PR, C], bf16, tag="mct")
                for pr in range(PR):
                    mc_ps = mm_pool.tile([128, C], f32, tag="mm")
                    nc.tensor.matmul(
                        mc_ps[:], ctxt_sb[:, pr, :], wo[:, pr, :], start=True, stop=True
                    )
                    nc.scalar.activation(
                        mct[:, pr, 0:128],
                        mc_ps[:, 0:128],
                        Copy,
                        scale=inv_den[:, pr : pr + 1],
                    )
                    nc.vector.tensor_scalar_mul(
                        mct[:, pr, 128:256],
                        mc_ps[:, 128:256],
                        inv_den[:, pr : pr + 1],
                    )

                # fused output projection, per l-chunk of 512.  The very last
                # tile runs in 128-col pieces end-to-end so the drain tail
                # after the final matmul is short.
                for lt in range(LT):
                    tail = bi == NB - 1 and lt == LT - 1
                    ostg = ostp.tile([128, 2, 512], bf16, tag="ostg")
                    if not tail:
                        for oc2 in range(2):
                            psf = mm_pool.tile([128, 512], f32, tag="mm")
                            for pr in range(PR):
                                nc.tensor.matmul(
                                    psf[:],
                                    mct[:, pr, oc2 * 128 : (oc2 + 1) * 128],
                                    qt[:, pr, lt * 512 : (lt + 1) * 512],
                                    start=(pr == 0),
                                    stop=(pr == PR - 1),
                                )
                            nc.scalar.activation(
                                ostg[:, oc2, :],
                                psf[:],
                                Ident,
                                bias=bb[:, oc2 : oc2 + 1],
                            )
                        nc.sync.dma_start(
                            out_d[bi, :, :, lt * 512 : (lt + 1) * 512], ostg[:]
                        )
                    else:
                        for pc in range(4):
                            p0 = lt * 512 + pc * 128
                            for oc2 in range(2):
                                psf = mm_pool.tile([128, 128], f32, tag="mm")
                                for pr in range(PR):
                                    nc.tensor.matmul(
                                        psf[:],
                                        mct[:, pr, oc2 * 128 : (oc2 + 1) * 128],
                                        qt[:, pr, p0 : p0 + 128],
                                        start=(pr == 0),
                                        stop=(pr == PR - 1),
                                    )
                                nc.scalar.activation(
                                    ostg[:, oc2, pc * 128 : (pc + 1) * 128],
                                    psf[:],
                                    Ident,
                                    bias=bb[:, oc2 : oc2 + 1],
                                )
                            nc.sync.dma_start(
                                out_d[bi, :, :, p0 : p0 + 128],
                                ostg[:, :, pc * 128 : (pc + 1) * 128],
                            )

    nc.compile()
    return nc


def _get_nc():
    if "nc" not in _CACHE:
        _CACHE["nc"] = _build()
    return _CACHE["nc"]


def _prep_in_maps(x, w_qkv, w_out, b_out):
    import ml_dtypes

    bf16 = ml_dtypes.bfloat16

    def wt(w):  # (512, 256) slice -> [128, CC, HID] partition-major
        return np.ascontiguousarray(
            w.T.reshape(CC, 128, HID).transpose(1, 0, 2)
        ).astype(bf16)

    wq_t = wt(w_qkv[0:512])
    wk_t = wt(w_qkv[512:1024])
    wv_t = wt(w_qkv[1024:1536])
    wo_t = np.ascontiguousarray(
        w_out.T.reshape(PR, 128, C).transpose(1, 0, 2)
    ).astype(bf16)
    bb = np.ascontiguousarray(b_out.reshape(2, 128).T).astype(np.float32)
    in_maps = []
    for c in range(N_CORES):
        xs = (
            x[c * NB : (c + 1) * NB]
            .reshape(NB, CC, 128, L)
            .transpose(0, 2, 1, 3)
            .astype(bf16)
        )
        in_maps.append(
            {
                "x": np.ascontiguousarray(xs),
                "wq_t": wq_t,
                "wk_t": wk_t,
                "wv_t": wv_t,
                "wo_t": wo_t,
                "bb": bb,
            }
        )
    return in_maps


def kernel(x, w_qkv, w_out, b_out):
    from concourse.bass_utils import run_bass_kernel_spmd

    nc = _get_nc()
    in_maps = _prep_in_maps(
        np.asarray(x, dtype=np.float32),
        np.asarray(w_qkv, dtype=np.float32),
        np.asarray(w_out, dtype=np.float32),
        np.asarray(b_out, dtype=np.float32),
    )
    res = run_bass_kernel_spmd(nc, in_maps, core_ids=list(range(N_CORES)))
    # out is [NB, 128, 2, L]; channel c = oc2 * 128 + p
    out = np.concatenate(
        [
            res.results[c]["out"]
            .astype(np.float32)
            .transpose(0, 2, 1, 3)
            .reshape(NB, C, L)
            for c in range(N_CORES)
        ],
        axis=0,
    )
    return out


# revision 33
# speedup vs baseline: 4.5677x; 1.4983x over previous
"""LinearAttention Trainium2 kernel: data-parallel over batch on 8 NeuronCores.

Reference computation per batch b (C=256 channels, L=4096 seq, H=8 heads, D=64):
  qkv = w_qkv @ x[b]                    # (1536, L)
  q, k, v = split(qkv)                  # each (512, L), rows = (head, dim)
  k = softmax(k, axis=L)
  ctx[h] = k[h] @ v[h].T                # (64, 64)
  out[h] = ctx[h].T @ q[h]              # (64, L)
  y[b] = w_out @ concat(out) + b_out    # (256, L)

Per-core design (2 batches/core):
  - K^T, V^T computed with L on partitions (lhsT = x chunk, rhs = w^T) so the
    context matmul contracts over L on the TensorEngine.
  - context computed TRANSPOSED per head-pair: ctxT[e,d] = sum_l v[e,l]exp(k[d,l])
    (lhsT = v^T chunk, rhs = expk^T chunk), cross-head quadrants discarded via
    a zeroed block-diagonal SBUF tile.
  - softmax denominator via transposed N=1 matmuls per l-tile:
    den[d,1] += expkT_chunk^T @ ones — d lands on partitions directly and each
    matmul runs at the ~60-cycle issue floor instead of streaming 512 columns.
  - PSUM accumulation-group discipline: the has_written clear on start=True is
    bank-wide, so any bank hosting several interleaved accumulation groups
    (the four ctx tiles in one bank, the four den columns in another) is
    opened by a single zero-writing matmul; the real groups then accumulate
    with start=False, WAW-ordered after the clear.
  - two PSUM pools ping-pong between batches so one batch's K/V projections
    can overlap the previous batch's output phase on the PE.
  - ASSOCIATIVITY FOLD: y_attn = M^T q with q = Wq x collapses to
    y = (M^T Wq) x.  M = w_out-folded, 1/den-scaled context (McT[d,o],
    per-batch).  W_effT[c,o] = sum_d wq[d,c] McT[d,o] costs 8 N=256 matmuls;
    the output phase is then ONE 256-contraction GEMM over x (32 matmuls vs
    128 for separate Q + output projections — saves ~20us/batch of PE).
  - McT[d,o] = sum_e ctxT[e,d] wo[e,o], scaled by 1/den[d] on the PSUM->SBUF
    copy (split per 128-col half across ACT and DVE).
  - output stored as bf16 (well within tolerance, halves store traffic);
    bias applied on the PSUM->SBUF copy, alternating ACT (activation+bias)
    and DVE (tensor_scalar_add) per channel half.
  - x DMA is issued in chunks with the first 512 columns ahead of most weight
    loads; the very last output tile runs in 128-col pieces end-to-end so the
    drain tail after the final matmul is short.
  - exp() applied unshifted (inputs are N(0,1)-scaled; max |k| ~ 5, safe in f32).
  - all TensorE compute in bf16 (f32 PSUM accumulation).
"""

import numpy as np

B, C, L = 16, 256, 4096
HID = 512
N_CORES = 8
NB = B // N_CORES  # batches per core
CC = C // 128  # contraction chunks for the input projections (2)
LP = L // 128  # l-tiles with l on partitions (32)
LT = L // 512  # l-tiles of 512 for moving-dim matmuls (8)
PR = HID // 128  # head-pairs (4): each 128-wide chunk = 2 heads of 64
X_CHUNKS = (512, 512, 1024, 1024, 1024)  # x DMA chunk widths per batch

_CACHE = {}


def _build(reps=1):
    from concourse import bacc, mybir, tile

    bf16 = mybir.dt.bfloat16
    f32 = mybir.dt.float32
    Exp = mybir.ActivationFunctionType.Exp
    Copy = mybir.ActivationFunctionType.Copy
    Ident = mybir.ActivationFunctionType.Identity

    nc = bacc.Bacc(
        "TRN2",
        target_bir_lowering=False,
        debug=False,
        enable_asserts=False,
        num_devices=N_CORES,
    )

    # DRAM layouts put the partition dim first so whole-tensor DMAs iterate
    # in the same (partition-major) order as their SBUF-side APs.
    x_d = nc.dram_tensor("x", [NB, 128, CC, L], bf16, kind="ExternalInput")
    # wq in ORIGINAL [d, c] orientation (for the W_eff fold), d partition-major
    wq_d = nc.dram_tensor("wq_dc", [128, PR, C], bf16, kind="ExternalInput")
    wk_d = nc.dram_tensor("wk_t", [128, CC, HID], bf16, kind="ExternalInput")
    wv_d = nc.dram_tensor("wv_t", [128, CC, HID], bf16, kind="ExternalInput")
    wo_d = nc.dram_tensor("wo_t", [128, PR, C], bf16, kind="ExternalInput")
    bb_d = nc.dram_tensor("bb", [128, 2], f32, kind="ExternalInput")
    out_d = nc.dram_tensor("out", [NB, 128, 2, L], bf16, kind="ExternalOutput")

    with tile.TileContext(nc) as tc:
        with (
            tc.tile_pool(name="const", bufs=1) as const,
            tc.tile_pool(name="xp", bufs=2) as xp,
            tc.tile_pool(name="big", bufs=1) as big,
            tc.tile_pool(name="small", bufs=2) as small,
            tc.tile_pool(name="ostp", bufs=3) as ostp,
            tc.tile_pool(name="ps_a", bufs=3, space="PSUM") as ps_a,
            tc.tile_pool(name="ps_b", bufs=3, space="PSUM") as ps_b,
            tc.tile_pool(name="ps_ctx", bufs=1, space="PSUM") as ps_ctx,
            tc.tile_pool(name="ps_den", bufs=1, space="PSUM") as ps_den,
        ):
            wq = const.tile([128, PR, C], bf16)
            wk = const.tile([128, CC, HID], bf16)
            wv = const.tile([128, CC, HID], bf16)
            wo = const.tile([128, PR, C], bf16)
            bb = const.tile([128, 2], f32)
            ones_col = const.tile([128, 1], bf16)
            zeros128 = const.tile([128, 128], bf16)
            ctxt_sb = const.tile([128, PR, 128], bf16)

            # Startup DMA order follows first-use time on the serial SP queue;
            # wq/wo are first needed ~40us in (context fold), so they go last.
            xt0 = xp.tile([128, CC, L], bf16, name="xt_first")
            nc.sync.dma_start(xt0[:, :, 0:512], x_d[0, :, :, 0:512])
            nc.sync.dma_start(wk[:], wk_d[:])
            nc.sync.dma_start(wv[:], wv_d[:])
            nc.sync.dma_start(xt0[:, :, 512:1024], x_d[0, :, :, 512:1024])
            nc.sync.dma_start(xt0[:, :, 1024:2048], x_d[0, :, :, 1024:2048])
            nc.sync.dma_start(bb[:], bb_d[:])
            nc.sync.dma_start(xt0[:, :, 2048:3072], x_d[0, :, :, 2048:3072])
            nc.sync.dma_start(xt0[:, :, 3072:4096], x_d[0, :, :, 3072:4096])
            nc.sync.dma_start(wo[:], wo_d[:])
            nc.sync.dma_start(wq[:], wq_d[:])
            nc.gpsimd.memset(ones_col[:], 1.0)
            nc.gpsimd.memset(zeros128[:], 0.0)
            nc.gpsimd.memset(ctxt_sb[:], 0.0)

            def emit_x_dma(xt, bi):
                l0 = 0
                for cw in X_CHUNKS:
                    nc.sync.dma_start(
                        xt[:, :, l0 : l0 + cw], x_d[bi, :, :, l0 : l0 + cw]
                    )
                    l0 += cw

            for rep in range(reps):
              for bi in range(NB):
                first = rep == 0 and bi == 0
                mm_pool = ps_a if (rep * NB + bi) % 2 == 0 else ps_b
                xt = xt0 if first else xp.tile([128, CC, L], bf16)
                if not first:
                    emit_x_dma(xt, bi)

                expkt = big.tile([128, LP, HID], bf16, tag="expkt")
                vt = big.tile([128, LP, HID], bf16, tag="vt")
                den_acc = small.tile([128, HID], bf16, tag="denacc")

                # all four ctx tiles live in ONE PSUM bank; zero-writing
                # matmuls open the bank (bank-wide has_written clear on the
                # first), then the real groups accumulate with start=False.
                ctxb = ps_ctx.tile([128, PR, 128], f32, tag="ctx")
                den_ps = ps_den.tile([128, PR], f32, tag="den")
                for pr in range(PR):
                    nc.tensor.matmul(
                        ctxb[:, pr, :],
                        zeros128[:],
                        zeros128[:],
                        start=(pr == 0),
                        stop=True,
                        skip_group_check=True,
                    )
                nc.tensor.matmul(
                    den_ps[:], zeros128[:], zeros128[:, 0:PR], start=True, stop=True
                )
                for lp in range(LP):
                    psk = mm_pool.tile([128, HID], f32, tag="mm")
                    psv = mm_pool.tile([128, HID], f32, tag="mm")
                    for cc in range(CC):
                        nc.tensor.matmul(
                            psk[:],
                            xt[:, cc, lp * 128 : (lp + 1) * 128],
                            wk[:, cc, :],
                            start=(cc == 0),
                            stop=(cc == CC - 1),
                        )
                    for cc in range(CC):
                        nc.tensor.matmul(
                            psv[:],
                            xt[:, cc, lp * 128 : (lp + 1) * 128],
                            wv[:, cc, :],
                            start=(cc == 0),
                            stop=(cc == CC - 1),
                        )
                    nc.scalar.activation(expkt[:, lp, :], psk[:], Exp)
                    if lp < LP - 1:
                        nc.vector.tensor_copy(vt[:, lp, :], psv[:])
                    else:
                        # last tile in 128-col pieces: shortens the gate on the
                        # trailing context matmuls.
                        for pr in range(PR):
                            nc.vector.tensor_copy(
                                vt[:, lp, pr * 128 : (pr + 1) * 128],
                                psv[:, pr * 128 : (pr + 1) * 128],
                            )
                    for pr in range(PR):
                        nc.tensor.matmul(
                            ctxb[:, pr, :],
                            vt[:, lp, pr * 128 : (pr + 1) * 128],
                            expkt[:, lp, pr * 128 : (pr + 1) * 128],
                            start=False,
                            stop=(lp == LP - 1),
                            skip_group_check=True,
                        )
                    if lp == 0:
                        nc.vector.tensor_copy(den_acc[:], expkt[:, 0, :])
                    else:
                        nc.vector.tensor_add(den_acc[:], den_acc[:], expkt[:, lp, :])

                for pr in range(PR):
                    nc.tensor.matmul(
                        den_ps[:, pr : pr + 1],
                        den_acc[:, pr * 128 : (pr + 1) * 128],
                        ones_col[:],
                        start=False,
                        stop=True,
                        skip_group_check=True,
                    )
                inv_den = small.tile([128, PR], f32, tag="invden")
                nc.vector.reciprocal(inv_den[:], den_ps[:])

                # block-diagonal ctxT (cross-head quadrants stay zero); the two
                # diagonal blocks per pair go to different engines in parallel.
                for pr in range(PR):
                    nc.vector.tensor_copy(
                        ctxt_sb[0:64, pr, 0:64], ctxb[0:64, pr, 0:64]
                    )
                    nc.scalar.activation(
                        ctxt_sb[64:128, pr, 64:128], ctxb[64:128, pr, 64:128], Copy
                    )

                # fold w_out into the context: McT[d, o], scaled by 1/den[d].
                mct = small.tile([128, PR, C], bf16, tag="mct")
                for pr in range(PR):
                    mc_ps = mm_pool.tile([128, C], f32, tag="mm")
                    nc.tensor.matmul(
                        mc_ps[:], ctxt_sb[:, pr, :], wo[:, pr, :], start=True, stop=True
                    )
                    nc.scalar.activation(
                        mct[:, pr, 0:128],
                        mc_ps[:, 0:128],
                        Copy,
                        scale=inv_den[:, pr : pr + 1],
                    )
                    nc.vector.tensor_scalar_mul(
                        mct[:, pr, 128:256],
                        mc_ps[:, 128:256],
                        inv_den[:, pr : pr + 1],
                    )

                # fold wq as well: W_effT[c, o] = sum_d wq[d, c] McT[d, o].
                # The whole output phase becomes y = W_effT^T x + b.
                weff = small.tile([128, CC, C], bf16, tag="weff")
                for cc2 in range(CC):
                    psw = mm_pool.tile([128, C], f32, tag="mm")
                    for pr in range(PR):
                        nc.tensor.matmul(
                            psw[:],
                            wq[:, pr, cc2 * 128 : (cc2 + 1) * 128],
                            mct[:, pr, :],
                            start=(pr == 0),
                            stop=(pr == PR - 1),
                        )
                    if cc2 == 0:
                        nc.scalar.activation(weff[:, cc2, :], psw[:], Copy)
                    else:
                        nc.vector.tensor_copy(weff[:, cc2, :], psw[:])

                # output projection: single 256-contraction GEMM over x.
                # PSUM->SBUF copies alternate ACT (activation+bias) and DVE
                # (tensor_scalar_add with the per-partition bias column).
                for lt in range(LT):
                    tail = bi == NB - 1 and lt == LT - 1
                    ostg = ostp.tile([128, 2, 512], bf16, tag="ostg")
                    if not tail:
                        for oc2 in range(2):
                            psf = mm_pool.tile([128, 512], f32, tag="mm")
                            for cc in range(CC):
                                nc.tensor.matmul(
                                    psf[:],
                                    weff[:, cc, oc2 * 128 : (oc2 + 1) * 128],
                                    xt[:, cc, lt * 512 : (lt + 1) * 512],
                                    start=(cc == 0),
                                    stop=(cc == CC - 1),
                                )
                            if oc2 == 0:
                                nc.scalar.activation(
                                    ostg[:, oc2, :],
                                    psf[:],
                                    Ident,
                                    bias=bb[:, oc2 : oc2 + 1],
                                )
                            else:
                                nc.vector.tensor_scalar_add(
                                    ostg[:, oc2, :], psf[:], bb[:, oc2 : oc2 + 1]
                                )
                        nc.sync.dma_start(
                            out_d[bi, :, :, lt * 512 : (lt + 1) * 512], ostg[:]
                        )
                    else:
                        for pc in range(4):
                            p0 = lt * 512 + pc * 128
                            for oc2 in range(2):
                                psf = mm_pool.tile([128, 128], f32, tag="mm")
                                for cc in range(CC):
                                    nc.tensor.matmul(
                                        psf[:],
                                        weff[:, cc, oc2 * 128 : (oc2 + 1) * 128],
                                        xt[:, cc, p0 : p0 + 128],
                                        start=(cc == 0),
                                        stop=(cc == CC - 1),
                                    )
                                if oc2 == 0:
                                    nc.scalar.activation(
                                        ostg[:, oc2, pc * 128 : (pc + 1) * 128],
                                        psf[:],
                                        Ident,
                                        bias=bb[:, oc2 : oc2 + 1],
                                    )
                                else:
                                    nc.vector.tensor_scalar_add(
                                        ostg[:, oc2, pc * 128 : (pc + 1) * 128],
                                        psf[:],
                                        bb[:, oc2 : oc2 + 1],
                                    )
                            nc.sync.dma_start(
                                out_d[bi, :, :, p0 : p0 + 128],
                                ostg[:, :, pc * 128 : (pc + 1) * 128],
                            )

    nc.compile()
    return nc


def _get_nc():
    if "nc" not in _CACHE:
        _CACHE["nc"] = _build()
    return _CACHE["nc"]


def _prep_in_maps(x, w_qkv, w_out, b_out):
    import ml_dtypes

    bf16 = ml_dtypes.bfloat16

    def wt(w):  # (512, 256) slice -> [128, CC, HID] partition-major (w^T)
        return np.ascontiguousarray(
            w.T.reshape(CC, 128, HID).transpose(1, 0, 2)
        ).astype(bf16)

    # wq stays in [d, c] orientation for the W_eff fold
    wq_dc = np.ascontiguousarray(
        w_qkv[0:512].reshape(PR, 128, C).transpose(1, 0, 2)
    ).astype(bf16)
    wk_t = wt(w_qkv[512:1024])
    wv_t = wt(w_qkv[1024:1536])
    wo_t = np.ascontiguousarray(
        w_out.T.reshape(PR, 128, C).transpose(1, 0, 2)
    ).astype(bf16)
    bb = np.ascontiguousarray(b_out.reshape(2, 128).T).astype(np.float32)
    in_maps = []
    for c in range(N_CORES):
        xs = (
            x[c * NB : (c + 1) * NB]
            .reshape(NB, CC, 128, L)
            .transpose(0, 2, 1, 3)
            .astype(bf16)
        )
        in_maps.append(
            {
                "x": np.ascontiguousarray(xs),
                "wq_dc": wq_dc,
                "wk_t": wk_t,
                "wv_t": wv_t,
                "wo_t": wo_t,
                "bb": bb,
            }
        )
    return in_maps


def kernel(x, w_qkv, w_out, b_out):
    from concourse.bass_utils import run_bass_kernel_spmd

    nc = _get_nc()
    in_maps = _prep_in_maps(
        np.asarray(x, dtype=np.float32),
        np.asarray(w_qkv, dtype=np.float32),
        np.asarray(w_out, dtype=np.float32),
        np.asarray(b_out, dtype=np.float32),
    )
    res = run_bass_kernel_spmd(nc, in_maps, core_ids=list(range(N_CORES)))
    # out is [NB, 128, 2, L]; channel c = oc2 * 128 + p
    out = np.concatenate(
        [
            res.results[c]["out"]
            .astype(np.float32)
            .transpose(0, 2, 1, 3)
            .reshape(NB, C, L)
            for c in range(N_CORES)
        ],
        axis=0,
    )
    return out


# revision 37
# speedup vs baseline: 4.6698x; 1.0223x over previous
"""LinearAttention Trainium2 kernel: data-parallel over batch on 8 NeuronCores.

Reference computation per batch b (C=256 channels, L=4096 seq, H=8 heads, D=64):
  qkv = w_qkv @ x[b]                    # (1536, L)
  q, k, v = split(qkv)                  # each (512, L), rows = (head, dim)
  k = softmax(k, axis=L)
  ctx[h] = k[h] @ v[h].T                # (64, 64)
  out[h] = ctx[h].T @ q[h]              # (64, L)
  y[b] = w_out @ concat(out) + b_out    # (256, L)

Per-core design (2 batches/core):
  - K^T, V^T computed with L on partitions (lhsT = x chunk, rhs = w^T) so the
    context matmul contracts over L on the TensorEngine.
  - context computed TRANSPOSED per head-pair: ctxT[e,d] = sum_l v[e,l]exp(k[d,l])
    (lhsT = v^T chunk, rhs = expk^T chunk), cross-head quadrants discarded via
    a zeroed block-diagonal SBUF tile.
  - softmax denominator: exp(k) tiles accumulate elementwise on the DVE (bf16,
    per-partition rounding averages out over the final 128-partition sum),
    then four transposed N=1 matmuls reduce partitions — d lands on the
    partition dim directly, no row-transpose chain and no per-tile PE work.
  - PSUM accumulation-group discipline: the has_written clear on start=True is
    bank-wide, so any bank hosting several interleaved accumulation groups
    (the four ctx tiles in one bank, the four den columns in another) is
    opened by a single zero-writing matmul; the real groups then accumulate
    with start=False, WAW-ordered after the clear.
  - two PSUM pools ping-pong between batches so one batch's K/V projections
    can overlap the previous batch's output phase on the PE.
  - ASSOCIATIVITY FOLD: y_attn = M^T q with q = Wq x collapses to
    y = (M^T Wq) x.  M = w_out-folded, 1/den-scaled context (McT[d,o],
    per-batch).  W_effT[c,o] = sum_d wq[d,c] McT[d,o] costs 8 N=256 matmuls;
    the output phase is then ONE 256-contraction GEMM over x (32 matmuls vs
    128 for separate Q + output projections — saves ~20us/batch of PE).
  - McT[d,o] = sum_e ctxT[e,d] wo[e,o], scaled by 1/den[d] on the PSUM->SBUF
    copy (split per 128-col half across ACT and DVE).
  - output stored as bf16 (well within tolerance, halves store traffic);
    bias applied on the PSUM->SBUF copy, alternating ACT (activation+bias)
    and DVE (tensor_scalar_add) per channel half.
  - x DMA is issued in chunks with the first 512 columns ahead of most weight
    loads; the very last output tile runs in 128-col pieces end-to-end so the
    drain tail after the final matmul is short.
  - exp() applied unshifted (inputs are N(0,1)-scaled; max |k| ~ 5, safe in f32).
  - all TensorE compute in bf16 (f32 PSUM accumulation).
"""

import numpy as np

B, C, L = 16, 256, 4096
HID = 512
N_CORES = 8
NB = B // N_CORES  # batches per core
CC = C // 128  # contraction chunks for the input projections (2)
LP = L // 128  # l-tiles with l on partitions (32)
LT = L // 512  # l-tiles of 512 for moving-dim matmuls (8)
PR = HID // 128  # head-pairs (4): each 128-wide chunk = 2 heads of 64
X_CHUNKS = (512, 512, 1024, 1024, 1024)  # x DMA chunk widths per batch

_CACHE = {}


def _build(reps=1):
    from concourse import bacc, mybir, tile

    bf16 = mybir.dt.bfloat16
    f32 = mybir.dt.float32
    Exp = mybir.ActivationFunctionType.Exp
    Copy = mybir.ActivationFunctionType.Copy
    Ident = mybir.ActivationFunctionType.Identity

    nc = bacc.Bacc(
        "TRN2",
        target_bir_lowering=False,
        debug=False,
        enable_asserts=False,
        num_devices=N_CORES,
    )

    # DRAM layouts put the partition dim first so whole-tensor DMAs iterate
    # in the same (partition-major) order as their SBUF-side APs.
    x_d = nc.dram_tensor("x", [NB, 128, CC, L], bf16, kind="ExternalInput")
    # wq in ORIGINAL [d, c] orientation (for the W_eff fold), d partition-major
    wq_d = nc.dram_tensor("wq_dc", [128, PR, C], bf16, kind="ExternalInput")
    wk_d = nc.dram_tensor("wk_t", [128, CC, HID], bf16, kind="ExternalInput")
    wv_d = nc.dram_tensor("wv_t", [128, CC, HID], bf16, kind="ExternalInput")
    wo_d = nc.dram_tensor("wo_t", [128, PR, C], bf16, kind="ExternalInput")
    bb_d = nc.dram_tensor("bb", [128, 2], f32, kind="ExternalInput")
    out_d = nc.dram_tensor("out", [NB, 128, 2, L], bf16, kind="ExternalOutput")

    with tile.TileContext(nc) as tc:
        with (
            tc.tile_pool(name="const", bufs=1) as const,
            tc.tile_pool(name="xp", bufs=2) as xp,
            tc.tile_pool(name="big", bufs=1) as big,
            tc.tile_pool(name="small", bufs=2) as small,
            tc.tile_pool(name="ostp", bufs=3) as ostp,
            tc.tile_pool(name="ps_a", bufs=3, space="PSUM") as ps_a,
            tc.tile_pool(name="ps_b", bufs=3, space="PSUM") as ps_b,
            tc.tile_pool(name="ps_ctx", bufs=1, space="PSUM") as ps_ctx,
            tc.tile_pool(name="ps_den", bufs=1, space="PSUM") as ps_den,
        ):
            wq = const.tile([128, PR, C], bf16)
            wk = const.tile([128, CC, HID], bf16)
            wv = const.tile([128, CC, HID], bf16)
            wo = const.tile([128, PR, C], bf16)
            bb = const.tile([128, 2], f32)
            ones_col = const.tile([128, 1], bf16)
            zeros128 = const.tile([128, 128], bf16)
            ctxt_sb = const.tile([128, PR, 128], bf16)

            # Startup DMA order follows first-use time on the serial SP queue;
            # wq/wo are first needed ~40us in (context fold), so they go last.
            xt0 = xp.tile([128, CC, L], bf16, name="xt_first")
            nc.sync.dma_start(xt0[:, :, 0:512], x_d[0, :, :, 0:512])
            nc.sync.dma_start(wk[:], wk_d[:])
            nc.sync.dma_start(wv[:], wv_d[:])
            nc.sync.dma_start(xt0[:, :, 512:1024], x_d[0, :, :, 512:1024])
            nc.sync.dma_start(xt0[:, :, 1024:2048], x_d[0, :, :, 1024:2048])
            nc.sync.dma_start(bb[:], bb_d[:])
            nc.sync.dma_start(xt0[:, :, 2048:3072], x_d[0, :, :, 2048:3072])
            nc.sync.dma_start(xt0[:, :, 3072:4096], x_d[0, :, :, 3072:4096])
            nc.sync.dma_start(wo[:], wo_d[:])
            nc.sync.dma_start(wq[:], wq_d[:])
            nc.gpsimd.memset(ones_col[:], 1.0)
            nc.gpsimd.memset(zeros128[:], 0.0)
            nc.gpsimd.memset(ctxt_sb[:], 0.0)

            def emit_x_dma(xt, bi):
                l0 = 0
                for cw in X_CHUNKS:
                    nc.sync.dma_start(
                        xt[:, :, l0 : l0 + cw], x_d[bi, :, :, l0 : l0 + cw]
                    )
                    l0 += cw

            xts = {0: xt0}

            def get_xt(idx):
                if idx not in xts:
                    xts[idx] = xp.tile([128, CC, L], bf16, tag="xt", name=f"xt{idx}")
                return xts[idx]

            for rep in range(reps):
              for bi in range(NB):
                idx = rep * NB + bi
                mm_pool = ps_a if idx % 2 == 0 else ps_b
                xt = get_xt(idx)

                expkt = big.tile([128, LP, HID], bf16, tag="expkt")
                vt = big.tile([128, LP, HID], bf16, tag="vt")
                den_acc = small.tile([128, HID], bf16, tag="denacc")

                # all four ctx tiles live in ONE PSUM bank; zero-writing
                # matmuls open the bank (bank-wide has_written clear on the
                # first), then the real groups accumulate with start=False.
                ctxb = ps_ctx.tile([128, PR, 128], f32, tag="ctx")
                den_ps = ps_den.tile([128, PR], f32, tag="den")
                for pr in range(PR):
                    nc.tensor.matmul(
                        ctxb[:, pr, :],
                        zeros128[:],
                        zeros128[:],
                        start=(pr == 0),
                        stop=True,
                        skip_group_check=True,
                    )
                nc.tensor.matmul(
                    den_ps[:], zeros128[:], zeros128[:, 0:PR], start=True, stop=True
                )
                for lp in range(LP):
                    psk = mm_pool.tile([128, HID], f32, tag="mm")
                    psv = mm_pool.tile([128, HID], f32, tag="mm")
                    for cc in range(CC):
                        nc.tensor.matmul(
                            psk[:],
                            xt[:, cc, lp * 128 : (lp + 1) * 128],
                            wk[:, cc, :],
                            start=(cc == 0),
                            stop=(cc == CC - 1),
                        )
                    for cc in range(CC):
                        nc.tensor.matmul(
                            psv[:],
                            xt[:, cc, lp * 128 : (lp + 1) * 128],
                            wv[:, cc, :],
                            start=(cc == 0),
                            stop=(cc == CC - 1),
                        )
                    nc.scalar.activation(expkt[:, lp, :], psk[:], Exp)
                    if lp < LP - 1:
                        nc.vector.tensor_copy(vt[:, lp, :], psv[:])
                    else:
                        # last tile in 128-col pieces: shortens the gate on the
                        # trailing context matmuls.
                        for pr in range(PR):
                            nc.vector.tensor_copy(
                                vt[:, lp, pr * 128 : (pr + 1) * 128],
                                psv[:, pr * 128 : (pr + 1) * 128],
                            )
                    for pr in range(PR):
                        nc.tensor.matmul(
                            ctxb[:, pr, :],
                            vt[:, lp, pr * 128 : (pr + 1) * 128],
                            expkt[:, lp, pr * 128 : (pr + 1) * 128],
                            start=False,
                            stop=(lp == LP - 1),
                            skip_group_check=True,
                        )
                    if lp == 0:
                        nc.vector.tensor_copy(den_acc[:], expkt[:, 0, :])
                    else:
                        nc.vector.tensor_add(den_acc[:], den_acc[:], expkt[:, lp, :])

                for pr in range(PR):
                    nc.tensor.matmul(
                        den_ps[:, pr : pr + 1],
                        den_acc[:, pr * 128 : (pr + 1) * 128],
                        ones_col[:],
                        start=False,
                        stop=True,
                        skip_group_check=True,
                    )
                inv_den = small.tile([128, PR], f32, tag="invden")
                nc.vector.reciprocal(inv_den[:], den_ps[:])

                # block-diagonal ctxT (cross-head quadrants stay zero); the two
                # diagonal blocks per pair go to different engines in parallel.
                for pr in range(PR):
                    nc.vector.tensor_copy(
                        ctxt_sb[0:64, pr, 0:64], ctxb[0:64, pr, 0:64]
                    )
                    nc.scalar.activation(
                        ctxt_sb[64:128, pr, 64:128], ctxb[64:128, pr, 64:128], Copy
                    )

                # fold w_out into the context: McT[d, o], scaled by 1/den[d].
                mct = small.tile([128, PR, C], bf16, tag="mct")
                for pr in range(PR):
                    mc_ps = mm_pool.tile([128, C], f32, tag="mm")
                    nc.tensor.matmul(
                        mc_ps[:], ctxt_sb[:, pr, :], wo[:, pr, :], start=True, stop=True
                    )
                    nc.scalar.activation(
                        mct[:, pr, 0:128],
                        mc_ps[:, 0:128],
                        Copy,
                        scale=inv_den[:, pr : pr + 1],
                    )
                    nc.vector.tensor_scalar_mul(
                        mct[:, pr, 128:256],
                        mc_ps[:, 128:256],
                        inv_den[:, pr : pr + 1],
                    )

                # fold wq as well: W_effT[c, o] = sum_d wq[d, c] McT[d, o].
                # The whole output phase becomes y = W_effT^T x + b.
                weff = small.tile([128, CC, C], bf16, tag="weff")
                for cc2 in range(CC):
                    psw = mm_pool.tile([128, C], f32, tag="mm")
                    for pr in range(PR):
                        nc.tensor.matmul(
                            psw[:],
                            wq[:, pr, cc2 * 128 : (cc2 + 1) * 128],
                            mct[:, pr, :],
                            start=(pr == 0),
                            stop=(pr == PR - 1),
                        )
                    if cc2 == 0:
                        nc.scalar.activation(weff[:, cc2, :], psw[:], Copy)
                    else:
                        nc.vector.tensor_copy(weff[:, cc2, :], psw[:])

                # prefetch the next batch's x ahead of this batch's output
                # stores so its first l-tiles don't queue behind them on SP.
                if idx + 1 < reps * NB:
                    emit_x_dma(get_xt(idx + 1), (bi + 1) % NB)

                # output projection: single 256-contraction GEMM over x.
                # PSUM->SBUF copies alternate ACT (activation+bias) and DVE
                # (tensor_scalar_add with the per-partition bias column);
                # one combined store per tile (serial-SP DMA floor makes
                # fewer, larger stores cheaper than piecewise ones).
                for lt in range(LT):
                    ostg = ostp.tile([128, 2, 512], bf16, tag="ostg")
                    for oc2 in range(2):
                        psf = mm_pool.tile([128, 512], f32, tag="mm")
                        for cc in range(CC):
                            nc.tensor.matmul(
                                psf[:],
                                weff[:, cc, oc2 * 128 : (oc2 + 1) * 128],
                                xt[:, cc, lt * 512 : (lt + 1) * 512],
                                start=(cc == 0),
                                stop=(cc == CC - 1),
                            )
                        if oc2 == 0:
                            nc.scalar.activation(
                                ostg[:, oc2, :],
                                psf[:],
                                Ident,
                                bias=bb[:, oc2 : oc2 + 1],
                            )
                        else:
                            nc.vector.tensor_scalar_add(
                                ostg[:, oc2, :], psf[:], bb[:, oc2 : oc2 + 1]
                            )
                    nc.sync.dma_start(
                        out_d[bi, :, :, lt * 512 : (lt + 1) * 512], ostg[:]
                    )

    nc.compile()
    return nc


def _get_nc():
    if "nc" not in _CACHE:
        _CACHE["nc"] = _build()
    return _CACHE["nc"]


def _prep_in_maps(x, w_qkv, w_out, b_out):
    import ml_dtypes

    bf16 = ml_dtypes.bfloat16

    def wt(w):  # (512, 256) slice -> [128, CC, HID] partition-major (w^T)
        return np.ascontiguousarray(
            w.T.reshape(CC, 128, HID).transpose(1, 0, 2)
        ).astype(bf16)

    # wq stays in [d, c] orientation for the W_eff fold
    wq_dc = np.ascontiguousarray(
        w_qkv[0:512].reshape(PR, 128, C).transpose(1, 0, 2)
    ).astype(bf16)
    wk_t = wt(w_qkv[512:1024])
    wv_t = wt(w_qkv[1024:1536])
    wo_t = np.ascontiguousarray(
        w_out.T.reshape(PR, 128, C).transpose(1, 0, 2)
    ).astype(bf16)
    bb = np.ascontiguousarray(b_out.reshape(2, 128).T).astype(np.float32)
    in_maps = []
    for c in range(N_CORES):
        xs = (
            x[c * NB : (c + 1) * NB]
            .reshape(NB, CC, 128, L)
            .transpose(0, 2, 1, 3)
            .astype(bf16)
        )
        in_maps.append(
            {
                "x": np.ascontiguousarray(xs),
                "wq_dc": wq_dc,
                "wk_t": wk_t,
                "wv_t": wv_t,
                "wo_t": wo_t,
                "bb": bb,
            }
        )
    return in_maps


def kernel(x, w_qkv, w_out, b_out):
    from concourse.bass_utils import run_bass_kernel_spmd

    nc = _get_nc()
    in_maps = _prep_in_maps(
        np.asarray(x, dtype=np.float32),
        np.asarray(w_qkv, dtype=np.float32),
        np.asarray(w_out, dtype=np.float32),
        np.asarray(b_out, dtype=np.float32),
    )
    res = run_bass_kernel_spmd(nc, in_maps, core_ids=list(range(N_CORES)))
    # out is [NB, 128, 2, L]; channel c = oc2 * 128 + p
    out = np.concatenate(
        [
            res.results[c]["out"]
            .astype(np.float32)
            .transpose(0, 2, 1, 3)
            .reshape(NB, C, L)
            for c in range(N_CORES)
        ],
        axis=0,
    )
    return out
